# revision 21
# baseline (speedup 1.0000x reference)
"""Trainium2 Bass kernel for single-head attention (B=4, S=4096, C=D=512).

Sharding: 8 cores = 4 batches x 2 query-halves. Each core:
  - projects K/V for its batch's full 4096 keys (duplicated across the
    two cores of a batch pair -- cheap vs. the O(S^2) attention),
  - projects Q for its own 2048 query rows,
  - computes softmax(Q K^T / sqrt(D)) V and the output projection for
    its 2048 rows.
No collectives; pure SPMD with per-core input slices.

On-chip layout notes:
  - All matmul contractions need the contracted dim on partitions, so x
    is transposed on the PE (128x128 transposes) into xT[d, rows].
  - Scores are computed transposed (scoreT[s, q]) so exp(scoreT) feeds
    the attnT matmul directly with no per-tile transposes.
  - Row sums l[q] come from a ones-matmul (lhsT = ones[128,128]) which
    also gives the partition-broadcast layout needed to normalize attnT.
  - Matmuls run in float32r (reduced-precision fp32, 1 cycle/row for
    N>=256 vs 4 for fp32). The BIR verifier requires f32r matmul inputs
    to be produced rounded-to-f32r, so every copyback writes f32r tiles.
"""

import sys

for _p in ("/opt/trn_rl_repo", "/root/.axon_site/_ro/trn_rl_repo"):
    if _p not in sys.path:
        sys.path.append(_p)

import numpy as np
import concourse.bacc as bacc
import concourse.mybir as mybir
import concourse.tile as tile
from concourse.bass_utils import run_bass_kernel_spmd
from concourse.masks import make_identity

F32 = mybir.dt.float32
F32R = mybir.dt.float32r
BF16 = mybir.dt.bfloat16

# matmul operand dtype: BF16 (1 cyc/row, fast FWL weight loads, half SBUF)
# or F32R (reduced fp32: ~2 cyc/row, 2x accuracy headroom)
import os
_MODE = os.environ.get("ATTN_MM_DT", "bf16")
# MM_DT: dtype for V/pT/attnT/weights; KQ_DT: dtype for the Q/K tensors
# feeding the score matmul (exp amplifies score error, so it gets priority
# for extra precision in "mixed" mode).
MM_DT = {"bf16": BF16, "f32r": F32R, "mixed": BF16}[_MODE]
KQ_DT = {"bf16": BF16, "f32r": F32R, "mixed": F32R}[_MODE]

B, S, C, D = 4, 4096, 512, 512
Q = S // 2          # queries per core
N_CORES = 8
SCALE = float(D) ** -0.5
QB = 512            # query block (psum bank width in fp32)
N_QB = Q // QB      # 4 query blocks per core
N_ST = S // 128     # 32 key tiles
N_DC = C // 128     # 4 contraction chunks


def _build_program():
    nc = bacc.Bacc(None, target_bir_lowering=False, debug=False)

    x = nc.dram_tensor("x", [S, C], F32, kind="ExternalInput")
    xq = nc.dram_tensor("xq", [Q, C], F32, kind="ExternalInput")
    w_dram = {
        name: nc.dram_tensor(name, [C, D], F32, kind="ExternalInput")
        for name in ("Wq", "Wk", "Wv", "Wo")
    }
    b_dram = {
        name: nc.dram_tensor(name, [D], F32, kind="ExternalInput")
        for name in ("bq", "bk", "bv", "bo")
    }
    out = nc.dram_tensor("out", [Q, D], F32, kind="ExternalOutput")

    ActFn = mybir.ActivationFunctionType

    with tile.TileContext(nc) as tc:
        persist = tc.alloc_tile_pool(name="persist", bufs=1)
        const = tc.alloc_tile_pool(name="const", bufs=1)
        xqload = tc.alloc_tile_pool(name="xqload", bufs=8)
        wkv_pool = tc.alloc_tile_pool(name="wkv", bufs=1)
        wraw_pool = tc.alloc_tile_pool(name="wraw", bufs=2)

        # ---- constants needed by the first transposes ----
        identity = const.tile([128, 128], F32, tag="identity")
        make_identity(nc, identity[:])
        ones_f32 = const.tile([128, 128], F32, tag="ones_f32")
        nc.vector.memset(ones_f32[:], 1.0)

        wts = {}

        def emit_weight(name):
            pool = wkv_pool if name in ("Wk", "Wv") else persist
            wt = pool.tile([128, N_DC, D], MM_DT, tag=f"w_{name}", name=f"w_{name}")
            for dc in range(N_DC):
                raw = wraw_pool.tile([128, D], F32, tag="raw", name=f"raw_{name}{dc}")
                nc.sync.dma_start(raw[:], w_dram[name][dc * 128 : (dc + 1) * 128, :])
                nc.vector.tensor_copy(wt[:, dc, :], raw[:])
            wts[name] = wt

        # DMA priority: Wk, then the first x row-groups, then everything else.
        emit_weight("Wk")

        # ---- persistent activations ----
        kT = persist.tile([128, N_DC, S], KQ_DT, tag="kT")     # kT[p, dc, s] = K[s, dc*128+p]
        vv = persist.tile([128, N_ST, D], MM_DT, tag="v")      # vv[p, i, e] = V[i*128+p, e]

        # ================= phase A: K/V projections =================
        # x-transposes run two row-groups ahead of the projection matmuls.
        xload = tc.alloc_tile_pool(name="xload", bufs=12)
        xTgp = tc.alloc_tile_pool(name="xTg", bufs=3)
        ps_tr = tc.alloc_tile_pool(name="ps_tr", bufs=3, space="PSUM")
        ps_proj = tc.alloc_tile_pool(name="ps_proj", bufs=3, space="PSUM")

        def emit_xT(src_dram, rg, tagpfx="", preloaded=None, pt_tag="pt"):
            xTg = xTgp.tile([128, N_DC, 512], MM_DT, tag="xTg", name=f"{tagpfx}xTg{rg}")
            for rt in range(4):
                if preloaded is not None:
                    xt = preloaded[rt]
                else:
                    xt = xload.tile([128, C], F32, tag="xt", name=f"{tagpfx}xt{rg}_{rt}")
                    nc.sync.dma_start(xt[:], src_dram[(rg * 4 + rt) * 128 : (rg * 4 + rt + 1) * 128, :])
                pt = ps_tr.tile([128, 512], F32, tag=pt_tag, name=f"{tagpfx}pt{rg}_{rt}")
                for dc in range(N_DC):
                    nc.tensor.transpose(pt[:, dc * 128 : (dc + 1) * 128],
                                        xt[:, dc * 128 : (dc + 1) * 128], identity[:])
                nc.vector.tensor_copy(
                    xTg[:, :, rt * 128 : rt * 128 + 128],
                    pt[:].rearrange("p (a b) -> p a b", a=N_DC),
                )
            return xTg

        # pre-load the ACT exp table during phase-A DMA waits
        warm = const.tile([1, 1], F32, tag="warm")
        nc.scalar.activation(warm[:], ones_f32[0:1, 0:1], ActFn.Exp, scale=1.0)

        pipe = [emit_xT(x, 0)]

        # prefetch the first query block's rows so phase B starts instantly
        xq0_tiles = []
        for rt in range(4):
            xt = xqload.tile([128, C], F32, tag="xqt", name=f"pre_xqt{rt}")
            nc.sync.dma_start(xt[:], xq[rt * 128 : (rt + 1) * 128, :])
            xq0_tiles.append(xt)

        # remaining constants + weights (queue behind the first x loads)
        emit_weight("Wv")
        ones_r = const.tile([128, 128], MM_DT, tag="ones_r")
        nc.vector.tensor_copy(ones_r[:], ones_f32[:])
        bqT = const.tile([128, N_DC], F32, tag="bqT")
        bkT = const.tile([128, N_DC], F32, tag="bkT")
        for g in range(N_DC):
            nc.sync.dma_start(bqT[:, g : g + 1], b_dram["bq"][g * 128 : (g + 1) * 128].unsqueeze(1))
            nc.sync.dma_start(bkT[:, g : g + 1], b_dram["bk"][g * 128 : (g + 1) * 128].unsqueeze(1))
        bias_bc = const.tile([128, 2 * D], F32, tag="bias_bc")
        with tc.tile_pool(name="browp", bufs=1) as browp, \
             tc.tile_pool(name="bcast_ps", bufs=1, space="PSUM") as bps:
            brow = browp.tile([1, 2 * D], F32, tag="brow")
            nc.sync.dma_start(brow[0:1, 0:D], b_dram["bv"][:].unsqueeze(0))
            nc.sync.dma_start(brow[0:1, D : 2 * D], b_dram["bo"][:].unsqueeze(0))
            bt = bps.tile([128, 512], F32)
            nc.tensor.matmul(bt[:], ones_f32[0:1, :], brow[0:1, 0:D])
            nc.vector.tensor_copy(bias_bc[:, 0:D], bt[:])
            bt2 = bps.tile([128, 512], F32)
            nc.tensor.matmul(bt2[:], ones_f32[0:1, :], brow[0:1, D : 2 * D])
            nc.vector.tensor_copy(bias_bc[:, D : 2 * D], bt2[:])
        bv_bc = bias_bc[:, 0:D]
        bo_bc = bias_bc[:, D : 2 * D]

        pipe.append(emit_xT(x, 1))

        N_RG = S // 512
        for rg in range(N_RG):              # 8 row groups of 512 rows
            xTg = pipe.pop(0)
            if rg + 2 < N_RG:
                pipe.append(emit_xT(x, rg + 2))
            # kT for these 512 rows
            for g in range(N_DC):
                pk = ps_proj.tile([128, 512], F32, tag="pk")
                for dc in range(N_DC):
                    nc.tensor.matmul(pk[:], wts["Wk"][:, dc, g * 128 : (g + 1) * 128],
                                     xTg[:, dc, :], start=(dc == 0), stop=(dc == N_DC - 1))
                nc.vector.tensor_scalar_add(kT[:, g, rg * 512 : (rg + 1) * 512], pk[:],
                                            bkT[:, g : g + 1])
            # V for these 512 rows
            for rt in range(4):
                pv = ps_proj.tile([128, 512], F32, tag="pk")
                for dc in range(N_DC):
                    nc.tensor.matmul(pv[:], xTg[:, dc, rt * 128 : (rt + 1) * 128],
                                     wts["Wv"][:, dc, :], start=(dc == 0), stop=(dc == N_DC - 1))
                nc.vector.tensor_add(vv[:, rg * 4 + rt, :], pv[:], bv_bc)
            if rg == 0:
                emit_weight("Wq")
                emit_weight("Wo")

        ps_proj.release(); ps_tr.release()
        xTgp.release(); xload.release()
        wraw_pool.release(); wkv_pool.release()

        # ================= phase B: attention =================
        with tc.tile_pool(name="xqTg", bufs=2) as xqTgp, \
             tc.tile_pool(name="qT", bufs=2) as qTp, \
             tc.tile_pool(name="pT", bufs=6) as pTp, \
             tc.tile_pool(name="rl", bufs=2) as rlp, \
             tc.tile_pool(name="attnT", bufs=2) as attnTp, \
             tc.tile_pool(name="osb", bufs=4) as osbp, \
             tc.tile_pool(name="ps_at", bufs=4, space="PSUM") as ps_atp, \
             tc.tile_pool(name="ps_s", bufs=3, space="PSUM") as ps_sp, \
             tc.tile_pool(name="ps_l", bufs=1, space="PSUM") as ps_lp:

            def emit_qproj(qb, preloaded=None):
                # Q projection for one 512-query block
                xqTg = emit_xT(xq, qb, tagpfx="q", preloaded=preloaded, pt_tag="ss")
                qT = qTp.tile([128, N_DC, 512], KQ_DT, tag="qT", name=f"qT{qb}")
                for g in range(N_DC):
                    pq = ps_sp.tile([128, 512], F32, tag="ss", name=f"pq{qb}_{g}")
                    for dc in range(N_DC):
                        nc.tensor.matmul(pq[:], wts["Wq"][:, dc, g * 128 : (g + 1) * 128],
                                         xqTg[:, dc, :], start=(dc == 0), stop=(dc == N_DC - 1))
                    nc.vector.tensor_scalar_add(qT[:, g, :], pq[:], bqT[:, g : g + 1])
                return qT

            def emit_score(qb, st, qT):
                ss = ps_sp.tile([128, 512], F32, tag="ss", name=f"ss{qb}_{st}")
                for dc in range(N_DC):
                    nc.tensor.matmul(ss[:], kT[:, dc, st * 128 : (st + 1) * 128],
                                     qT[:, dc, :], start=(dc == 0), stop=(dc == N_DC - 1))
                return ss

            # phase-B pools replace phase-A ones; re-point emit_xT pools
            xload = xqload
            xTgp = xqTgp
            ps_tr = ps_sp

            qT_cur = emit_qproj(0, preloaded=xq0_tiles)
            for qb in range(N_QB):
                qT = qT_cur
                l_ps = ps_lp.tile([128, 512], F32, tag="l", name=f"l{qb}")
                at_ps = [ps_atp.tile([128, 512], F32, tag="at", name=f"at{qb}_{et}")
                         for et in range(4)]
                ss_q = [emit_score(qb, 0, qT), emit_score(qb, 1, qT)]
                for st in range(N_ST):
                    if st + 2 < N_ST:
                        ss_q.append(emit_score(qb, st + 2, qT))
                    ss = ss_q.pop(0)
                    pT = pTp.tile([128, 512], MM_DT, tag="pT", name=f"pT{qb}_{st}")
                    nc.scalar.activation(pT[:], ss[:], ActFn.Exp, scale=SCALE)
                    nc.tensor.matmul(l_ps[:], ones_r[:], pT[:],
                                     start=(st == 0), stop=(st == N_ST - 1))
                    for et in range(4):
                        nc.tensor.matmul(at_ps[et][:], vv[:, st, et * 128 : (et + 1) * 128],
                                         pT[:], start=(st == 0), stop=(st == N_ST - 1))

                if qb + 1 < N_QB:
                    qT_cur = emit_qproj(qb + 1)

                # --- epilogue: 1/l applied per query row at the copyback ---
                l_row = rlp.tile([1, 512], F32, tag="l_row", name=f"lrow{qb}")
                nc.vector.tensor_copy(l_row[:], l_ps[0:1, :])
                lt_ps = ps_lp.tile([128, 4], F32, tag="l", name=f"lt{qb}")
                for rt in range(4):
                    nc.tensor.matmul(lt_ps[:, rt : rt + 1],
                                     l_row[0:1, rt * 128 : (rt + 1) * 128],
                                     ones_f32[0:1, 0:1])
                rlT = rlp.tile([128, 4], F32, tag="rlT", name=f"rlT{qb}")
                nc.vector.reciprocal(rlT[:], lt_ps[:])

                attnT = attnTp.tile([128, 4, 512], MM_DT, tag="attnT", name=f"attnT{qb}")
                for et in range(4):
                    nc.vector.tensor_copy(attnT[:, et, :], at_ps[et][:])
                for rt in range(4):
                    po = ps_sp.tile([128, 512], F32, tag="ss", name=f"po{qb}_{rt}")
                    for ec in range(4):
                        nc.tensor.matmul(po[:], attnT[:, ec, rt * 128 : (rt + 1) * 128],
                                         wts["Wo"][:, ec, :], start=(ec == 0), stop=(ec == 3))
                    ot = osbp.tile([128, D], F32, tag="ot", name=f"ot{qb}_{rt}")
                    # ot = po * (1/l[row])  [DVE], then + bo  [DVE]
                    nc.vector.tensor_scalar_mul(ot[:], po[:], rlT[:, rt : rt + 1])
                    nc.vector.tensor_add(ot[:], ot[:], bo_bc)
                    nc.sync.dma_start(out[(qb * 4 + rt) * 128 : (qb * 4 + rt + 1) * 128, :], ot[:])

        xqload.release()
        const.release()
        persist.release()

    nc.compile()
    return nc


_NC_CACHE = None


def _get_nc():
    global _NC_CACHE
    if _NC_CACHE is None:
        _NC_CACHE = _build_program()
    return _NC_CACHE


def kernel(**inputs):
    x = np.ascontiguousarray(np.asarray(inputs["x"], dtype=np.float32))
    xt = x.reshape(B, S, C)
    ws = {k: np.ascontiguousarray(np.asarray(inputs[k], dtype=np.float32))
          for k in ("Wq", "Wk", "Wv", "Wo")}
    bs = {k: np.ascontiguousarray(np.asarray(inputs[k], dtype=np.float32))
          for k in ("bq", "bk", "bv", "bo")}

    in_maps = []
    for c in range(N_CORES):
        b, h = divmod(c, 2)
        m = {"x": xt[b], "xq": np.ascontiguousarray(xt[b, h * Q : (h + 1) * Q])}
        m.update(ws)
        m.update(bs)
        in_maps.append(m)

    nc = _get_nc()
    res = run_bass_kernel_spmd(nc, in_maps, core_ids=list(range(N_CORES)))

    out = np.empty((B, S, D), dtype=np.float32)
    for c in range(N_CORES):
        b, h = divmod(c, 2)
        out[b, h * Q : (h + 1) * Q] = res.results[c]["out"]
    return out.reshape(B, 64, 64, D)


# revision 22
# speedup vs baseline: 1.0037x; 1.0037x over previous
"""Trainium2 Bass kernel for single-head attention (B=4, S=4096, C=D=512).

Sharding: 8 cores = 4 batches x 2 query-halves. Each core:
  - projects K/V for its batch's full 4096 keys (duplicated across the
    two cores of a batch pair -- cheap vs. the O(S^2) attention),
  - projects Q for its own 2048 query rows,
  - computes softmax(Q K^T / sqrt(D)) V and the output projection for
    its 2048 rows.
No collectives; pure SPMD with per-core input slices.

On-chip layout notes:
  - All matmul contractions need the contracted dim on partitions, so x
    is transposed on the PE (128x128 transposes) into xT[d, rows].
  - Scores are computed transposed (scoreT[s, q]) so exp(scoreT) feeds
    the attnT matmul directly with no per-tile transposes.
  - Row sums l[q] come from a ones-matmul (lhsT = ones[128,128]) which
    also gives the partition-broadcast layout needed to normalize attnT.
  - Matmuls run in float32r (reduced-precision fp32, 1 cycle/row for
    N>=256 vs 4 for fp32). The BIR verifier requires f32r matmul inputs
    to be produced rounded-to-f32r, so every copyback writes f32r tiles.
"""

import sys

for _p in ("/opt/trn_rl_repo", "/root/.axon_site/_ro/trn_rl_repo"):
    if _p not in sys.path:
        sys.path.append(_p)

import numpy as np
import concourse.bacc as bacc
import concourse.mybir as mybir
import concourse.tile as tile
from concourse.bass_utils import run_bass_kernel_spmd
from concourse.masks import make_identity

F32 = mybir.dt.float32
F32R = mybir.dt.float32r
BF16 = mybir.dt.bfloat16

# matmul operand dtype: BF16 (1 cyc/row, fast FWL weight loads, half SBUF)
# or F32R (reduced fp32: ~2 cyc/row, 2x accuracy headroom)
import os
_MODE = os.environ.get("ATTN_MM_DT", "bf16")
# MM_DT: dtype for V/pT/attnT/weights; KQ_DT: dtype for the Q/K tensors
# feeding the score matmul (exp amplifies score error, so it gets priority
# for extra precision in "mixed" mode).
MM_DT = {"bf16": BF16, "f32r": F32R, "mixed": BF16}[_MODE]
KQ_DT = {"bf16": BF16, "f32r": F32R, "mixed": F32R}[_MODE]

B, S, C, D = 4, 4096, 512, 512
Q = S // 2          # queries per core
N_CORES = 8
SCALE = float(D) ** -0.5
QB = 512            # query block (psum bank width in fp32)
N_QB = Q // QB      # 4 query blocks per core
N_ST = S // 128     # 32 key tiles
N_DC = C // 128     # 4 contraction chunks


def _build_program():
    nc = bacc.Bacc(None, target_bir_lowering=False, debug=False)

    x = nc.dram_tensor("x", [S, C], F32, kind="ExternalInput")
    xq = nc.dram_tensor("xq", [Q, C], F32, kind="ExternalInput")
    w_dram = {
        name: nc.dram_tensor(name, [C, D], F32, kind="ExternalInput")
        for name in ("Wq", "Wk", "Wv", "Wo")
    }
    b_dram = {
        name: nc.dram_tensor(name, [D], F32, kind="ExternalInput")
        for name in ("bq", "bk", "bv", "bo")
    }
    out = nc.dram_tensor("out", [Q, D], F32, kind="ExternalOutput")

    ActFn = mybir.ActivationFunctionType

    with tile.TileContext(nc) as tc:
        persist = tc.alloc_tile_pool(name="persist", bufs=1)
        const = tc.alloc_tile_pool(name="const", bufs=1)
        xqload = tc.alloc_tile_pool(name="xqload", bufs=8)
        wkv_pool = tc.alloc_tile_pool(name="wkv", bufs=1)
        wraw_pool = tc.alloc_tile_pool(name="wraw", bufs=2)

        # ---- constants needed by the first transposes ----
        identity = const.tile([128, 128], F32, tag="identity")
        make_identity(nc, identity[:])
        ones_f32 = const.tile([128, 128], F32, tag="ones_f32")
        nc.vector.memset(ones_f32[:], 1.0)

        wts = {}

        def emit_weight(name):
            pool = wkv_pool if name in ("Wk", "Wv") else persist
            wt = pool.tile([128, N_DC, D], MM_DT, tag=f"w_{name}", name=f"w_{name}")
            for dc in range(N_DC):
                raw = wraw_pool.tile([128, D], F32, tag="raw", name=f"raw_{name}{dc}")
                nc.sync.dma_start(raw[:], w_dram[name][dc * 128 : (dc + 1) * 128, :])
                nc.vector.tensor_copy(wt[:, dc, :], raw[:])
            wts[name] = wt

        # DMA priority: Wk, then the first x row-groups, then everything else.
        emit_weight("Wk")

        # ---- persistent activations ----
        kT = persist.tile([128, N_DC, S], KQ_DT, tag="kT")     # kT[p, dc, s] = K[s, dc*128+p]
        vv = persist.tile([128, N_ST, D], MM_DT, tag="v")      # vv[p, i, e] = V[i*128+p, e]

        # ================= phase A: K/V projections =================
        # x-transposes run two row-groups ahead of the projection matmuls.
        xload = tc.alloc_tile_pool(name="xload", bufs=16)
        xTgp = tc.alloc_tile_pool(name="xTg", bufs=3)
        ps_tr = tc.alloc_tile_pool(name="ps_tr", bufs=3, space="PSUM")
        ps_proj = tc.alloc_tile_pool(name="ps_proj", bufs=3, space="PSUM")

        def emit_xT(src_dram, rg, tagpfx="", preloaded=None, pt_tag="pt"):
            xTg = xTgp.tile([128, N_DC, 512], MM_DT, tag="xTg", name=f"{tagpfx}xTg{rg}")
            for rt in range(4):
                if preloaded is not None:
                    xt = preloaded[rt]
                else:
                    xt = xload.tile([128, C], F32, tag="xt", name=f"{tagpfx}xt{rg}_{rt}")
                    nc.sync.dma_start(xt[:], src_dram[(rg * 4 + rt) * 128 : (rg * 4 + rt + 1) * 128, :])
                pt = ps_tr.tile([128, 512], F32, tag=pt_tag, name=f"{tagpfx}pt{rg}_{rt}")
                for dc in range(N_DC):
                    nc.tensor.transpose(pt[:, dc * 128 : (dc + 1) * 128],
                                        xt[:, dc * 128 : (dc + 1) * 128], identity[:])
                nc.vector.tensor_copy(
                    xTg[:, :, rt * 128 : rt * 128 + 128],
                    pt[:].rearrange("p (a b) -> p a b", a=N_DC),
                )
            return xTg

        # pre-load the ACT exp table during phase-A DMA waits
        warm = const.tile([1, 1], F32, tag="warm")
        nc.scalar.activation(warm[:], ones_f32[0:1, 0:1], ActFn.Exp, scale=1.0)

        pipe = [emit_xT(x, 0)]

        # prefetch the first query block's rows so phase B starts instantly
        xq0_tiles = []
        for rt in range(4):
            xt = xqload.tile([128, C], F32, tag="xqt", name=f"pre_xqt{rt}")
            nc.sync.dma_start(xt[:], xq[rt * 128 : (rt + 1) * 128, :])
            xq0_tiles.append(xt)

        # remaining constants + weights (queue behind the first x loads)
        emit_weight("Wv")
        ones_r = const.tile([128, 128], MM_DT, tag="ones_r")
        nc.vector.tensor_copy(ones_r[:], ones_f32[:])
        bqT = const.tile([128, N_DC], F32, tag="bqT")
        bkT = const.tile([128, N_DC], F32, tag="bkT")
        for g in range(N_DC):
            nc.sync.dma_start(bqT[:, g : g + 1], b_dram["bq"][g * 128 : (g + 1) * 128].unsqueeze(1))
            nc.sync.dma_start(bkT[:, g : g + 1], b_dram["bk"][g * 128 : (g + 1) * 128].unsqueeze(1))
        bias_bc = const.tile([128, 2 * D], F32, tag="bias_bc")
        with tc.tile_pool(name="browp", bufs=1) as browp, \
             tc.tile_pool(name="bcast_ps", bufs=1, space="PSUM") as bps:
            brow = browp.tile([1, 2 * D], F32, tag="brow")
            nc.sync.dma_start(brow[0:1, 0:D], b_dram["bv"][:].unsqueeze(0))
            nc.sync.dma_start(brow[0:1, D : 2 * D], b_dram["bo"][:].unsqueeze(0))
            bt = bps.tile([128, 512], F32)
            nc.tensor.matmul(bt[:], ones_f32[0:1, :], brow[0:1, 0:D])
            nc.vector.tensor_copy(bias_bc[:, 0:D], bt[:])
            bt2 = bps.tile([128, 512], F32)
            nc.tensor.matmul(bt2[:], ones_f32[0:1, :], brow[0:1, D : 2 * D])
            nc.vector.tensor_copy(bias_bc[:, D : 2 * D], bt2[:])
        bv_bc = bias_bc[:, 0:D]
        bo_bc = bias_bc[:, D : 2 * D]

        pipe.append(emit_xT(x, 1))

        N_RG = S // 512
        for rg in range(N_RG):              # 8 row groups of 512 rows
            xTg = pipe.pop(0)
            if rg + 2 < N_RG:
                pipe.append(emit_xT(x, rg + 2))
            # kT for these 512 rows
            for g in range(N_DC):
                pk = ps_proj.tile([128, 512], F32, tag="pk")
                for dc in range(N_DC):
                    nc.tensor.matmul(pk[:], wts["Wk"][:, dc, g * 128 : (g + 1) * 128],
                                     xTg[:, dc, :], start=(dc == 0), stop=(dc == N_DC - 1))
                nc.vector.tensor_scalar_add(kT[:, g, rg * 512 : (rg + 1) * 512], pk[:],
                                            bkT[:, g : g + 1])
            # V for these 512 rows
            for rt in range(4):
                pv = ps_proj.tile([128, 512], F32, tag="pk")
                for dc in range(N_DC):
                    nc.tensor.matmul(pv[:], xTg[:, dc, rt * 128 : (rt + 1) * 128],
                                     wts["Wv"][:, dc, :], start=(dc == 0), stop=(dc == N_DC - 1))
                nc.vector.tensor_add(vv[:, rg * 4 + rt, :], pv[:], bv_bc)
            if rg == 0:
                emit_weight("Wq")
                emit_weight("Wo")

        ps_proj.release(); ps_tr.release()
        xTgp.release(); xload.release()
        wraw_pool.release(); wkv_pool.release()

        # ================= phase B: attention =================
        with tc.tile_pool(name="xqTg", bufs=2) as xqTgp, \
             tc.tile_pool(name="qT", bufs=2) as qTp, \
             tc.tile_pool(name="pT", bufs=6) as pTp, \
             tc.tile_pool(name="rl", bufs=2) as rlp, \
             tc.tile_pool(name="attnT", bufs=2) as attnTp, \
             tc.tile_pool(name="osb", bufs=4) as osbp, \
             tc.tile_pool(name="ps_at", bufs=4, space="PSUM") as ps_atp, \
             tc.tile_pool(name="ps_s", bufs=3, space="PSUM") as ps_sp, \
             tc.tile_pool(name="ps_l", bufs=1, space="PSUM") as ps_lp:

            def emit_qproj(qb, preloaded=None):
                # Q projection for one 512-query block
                xqTg = emit_xT(xq, qb, tagpfx="q", preloaded=preloaded, pt_tag="ss")
                qT = qTp.tile([128, N_DC, 512], KQ_DT, tag="qT", name=f"qT{qb}")
                for g in range(N_DC):
                    pq = ps_sp.tile([128, 512], F32, tag="ss", name=f"pq{qb}_{g}")
                    for dc in range(N_DC):
                        nc.tensor.matmul(pq[:], wts["Wq"][:, dc, g * 128 : (g + 1) * 128],
                                         xqTg[:, dc, :], start=(dc == 0), stop=(dc == N_DC - 1))
                    nc.vector.tensor_scalar_add(qT[:, g, :], pq[:], bqT[:, g : g + 1])
                return qT

            def emit_score(qb, st, qT):
                ss = ps_sp.tile([128, 512], F32, tag="ss", name=f"ss{qb}_{st}")
                for dc in range(N_DC):
                    nc.tensor.matmul(ss[:], kT[:, dc, st * 128 : (st + 1) * 128],
                                     qT[:, dc, :], start=(dc == 0), stop=(dc == N_DC - 1))
                return ss

            # phase-B pools replace phase-A ones; re-point emit_xT pools
            xload = xqload
            xTgp = xqTgp
            ps_tr = ps_sp

            qT_cur = emit_qproj(0, preloaded=xq0_tiles)
            for qb in range(N_QB):
                qT = qT_cur
                l_ps = ps_lp.tile([128, 512], F32, tag="l", name=f"l{qb}")
                at_ps = [ps_atp.tile([128, 512], F32, tag="at", name=f"at{qb}_{et}")
                         for et in range(4)]
                ss_q = [emit_score(qb, 0, qT), emit_score(qb, 1, qT)]
                for st in range(N_ST):
                    if st + 2 < N_ST:
                        ss_q.append(emit_score(qb, st + 2, qT))
                    if st == N_ST - 6 and qb + 1 < N_QB:
                        qT_cur = emit_qproj(qb + 1)
                    ss = ss_q.pop(0)
                    pT = pTp.tile([128, 512], MM_DT, tag="pT", name=f"pT{qb}_{st}")
                    nc.scalar.activation(pT[:], ss[:], ActFn.Exp, scale=SCALE)
                    nc.tensor.matmul(l_ps[:], ones_r[:], pT[:],
                                     start=(st == 0), stop=(st == N_ST - 1))
                    for et in range(4):
                        nc.tensor.matmul(at_ps[et][:], vv[:, st, et * 128 : (et + 1) * 128],
                                         pT[:], start=(st == 0), stop=(st == N_ST - 1))

                # --- epilogue: 1/l applied per query row at the copyback ---
                l_row = rlp.tile([1, 512], F32, tag="l_row", name=f"lrow{qb}")
                nc.vector.tensor_copy(l_row[:], l_ps[0:1, :])
                lt_ps = ps_lp.tile([128, 4], F32, tag="l", name=f"lt{qb}")
                for rt in range(4):
                    nc.tensor.matmul(lt_ps[:, rt : rt + 1],
                                     l_row[0:1, rt * 128 : (rt + 1) * 128],
                                     ones_f32[0:1, 0:1])
                rlT = rlp.tile([128, 4], F32, tag="rlT", name=f"rlT{qb}")
                nc.vector.reciprocal(rlT[:], lt_ps[:])

                attnT = attnTp.tile([128, 4, 512], MM_DT, tag="attnT", name=f"attnT{qb}")
                for et in range(4):
                    nc.vector.tensor_copy(attnT[:, et, :], at_ps[et][:])
                for rt in range(4):
                    po = ps_sp.tile([128, 512], F32, tag="ss", name=f"po{qb}_{rt}")
                    for ec in range(4):
                        nc.tensor.matmul(po[:], attnT[:, ec, rt * 128 : (rt + 1) * 128],
                                         wts["Wo"][:, ec, :], start=(ec == 0), stop=(ec == 3))
                    ot = osbp.tile([128, D], F32, tag="ot", name=f"ot{qb}_{rt}")
                    # ot = po * (1/l[row])  [DVE], then + bo  [DVE]
                    nc.vector.tensor_scalar_mul(ot[:], po[:], rlT[:, rt : rt + 1])
                    nc.vector.tensor_add(ot[:], ot[:], bo_bc)
                    nc.sync.dma_start(out[(qb * 4 + rt) * 128 : (qb * 4 + rt + 1) * 128, :], ot[:])

        xqload.release()
        const.release()
        persist.release()

    nc.compile()
    return nc


_NC_CACHE = None


def _get_nc():
    global _NC_CACHE
    if _NC_CACHE is None:
        _NC_CACHE = _build_program()
    return _NC_CACHE


def kernel(**inputs):
    x = np.ascontiguousarray(np.asarray(inputs["x"], dtype=np.float32))
    xt = x.reshape(B, S, C)
    ws = {k: np.ascontiguousarray(np.asarray(inputs[k], dtype=np.float32))
          for k in ("Wq", "Wk", "Wv", "Wo")}
    bs = {k: np.ascontiguousarray(np.asarray(inputs[k], dtype=np.float32))
          for k in ("bq", "bk", "bv", "bo")}

    in_maps = []
    for c in range(N_CORES):
        b, h = divmod(c, 2)
        m = {"x": xt[b], "xq": np.ascontiguousarray(xt[b, h * Q : (h + 1) * Q])}
        m.update(ws)
        m.update(bs)
        in_maps.append(m)

    nc = _get_nc()
    res = run_bass_kernel_spmd(nc, in_maps, core_ids=list(range(N_CORES)))

    out = np.empty((B, S, D), dtype=np.float32)
    for c in range(N_CORES):
        b, h = divmod(c, 2)
        out[b, h * Q : (h + 1) * Q] = res.results[c]["out"]
    return out.reshape(B, 64, 64, D)


# revision 23
# speedup vs baseline: 1.0090x; 1.0052x over previous
"""Trainium2 Bass kernel for single-head attention (B=4, S=4096, C=D=512).

Sharding: 8 cores = 4 batches x 2 query-halves. Each core:
  - projects K/V for its batch's full 4096 keys (duplicated across the
    two cores of a batch pair -- cheap vs. the O(S^2) attention),
  - projects Q for its own 2048 query rows,
  - computes softmax(Q K^T / sqrt(D)) V and the output projection for
    its 2048 rows.
No collectives; pure SPMD with per-core input slices.

On-chip layout notes:
  - All matmul contractions need the contracted dim on partitions, so x
    is transposed on the PE (128x128 transposes) into xT[d, rows].
  - Scores are computed transposed (scoreT[s, q]) so exp(scoreT) feeds
    the attnT matmul directly with no per-tile transposes.
  - Row sums l[q] come from a ones-matmul (lhsT = ones[128,128]) which
    also gives the partition-broadcast layout needed to normalize attnT.
  - Matmuls run in float32r (reduced-precision fp32, 1 cycle/row for
    N>=256 vs 4 for fp32). The BIR verifier requires f32r matmul inputs
    to be produced rounded-to-f32r, so every copyback writes f32r tiles.
"""

import sys

for _p in ("/opt/trn_rl_repo", "/root/.axon_site/_ro/trn_rl_repo"):
    if _p not in sys.path:
        sys.path.append(_p)

import numpy as np
import concourse.bacc as bacc
import concourse.mybir as mybir
import concourse.tile as tile
from concourse.bass_utils import run_bass_kernel_spmd
from concourse.masks import make_identity

F32 = mybir.dt.float32
F32R = mybir.dt.float32r
BF16 = mybir.dt.bfloat16

# matmul operand dtype: BF16 (1 cyc/row, fast FWL weight loads, half SBUF)
# or F32R (reduced fp32: ~2 cyc/row, 2x accuracy headroom)
import os
_MODE = os.environ.get("ATTN_MM_DT", "bf16")
# MM_DT: dtype for V/pT/attnT/weights; KQ_DT: dtype for the Q/K tensors
# feeding the score matmul (exp amplifies score error, so it gets priority
# for extra precision in "mixed" mode).
MM_DT = {"bf16": BF16, "f32r": F32R, "mixed": BF16}[_MODE]
KQ_DT = {"bf16": BF16, "f32r": F32R, "mixed": F32R}[_MODE]

B, S, C, D = 4, 4096, 512, 512
Q = S // 2          # queries per core
N_CORES = 8
SCALE = float(D) ** -0.5
QB = 512            # query block (psum bank width in fp32)
N_QB = Q // QB      # 4 query blocks per core
N_ST = S // 128     # 32 key tiles
N_DC = C // 128     # 4 contraction chunks


def _build_program():
    nc = bacc.Bacc(None, target_bir_lowering=False, debug=False)

    x = nc.dram_tensor("x", [S, C], F32, kind="ExternalInput")
    xq = nc.dram_tensor("xq", [Q, C], F32, kind="ExternalInput")
    w_dram = {
        name: nc.dram_tensor(name, [C, D], F32, kind="ExternalInput")
        for name in ("Wq", "Wk", "Wv", "Wo")
    }
    b_dram = {
        name: nc.dram_tensor(name, [D], F32, kind="ExternalInput")
        for name in ("bq", "bk", "bv", "bo")
    }
    out = nc.dram_tensor("out", [Q, D], F32, kind="ExternalOutput")

    ActFn = mybir.ActivationFunctionType

    with tile.TileContext(nc) as tc:
        persist = tc.alloc_tile_pool(name="persist", bufs=1)
        const = tc.alloc_tile_pool(name="const", bufs=1)
        xqload = tc.alloc_tile_pool(name="xqload", bufs=8)
        wkv_pool = tc.alloc_tile_pool(name="wkv", bufs=1)
        wraw_pool = tc.alloc_tile_pool(name="wraw", bufs=2)

        # ---- constants needed by the first transposes ----
        identity = const.tile([128, 128], F32, tag="identity")
        make_identity(nc, identity[:])
        ones_f32 = const.tile([128, 128], F32, tag="ones_f32")
        nc.vector.memset(ones_f32[:], 1.0)

        wts = {}

        def emit_weight(name):
            pool = wkv_pool if name in ("Wk", "Wv") else persist
            wt = pool.tile([128, N_DC, D], MM_DT, tag=f"w_{name}", name=f"w_{name}")
            for dc in range(N_DC):
                raw = wraw_pool.tile([128, D], F32, tag="raw", name=f"raw_{name}{dc}")
                nc.sync.dma_start(raw[:], w_dram[name][dc * 128 : (dc + 1) * 128, :])
                nc.vector.tensor_copy(wt[:, dc, :], raw[:])
            wts[name] = wt

        # DMA priority: Wk, then the first x row-groups, then everything else.
        emit_weight("Wk")

        # ---- persistent activations ----
        kT = persist.tile([128, N_DC, S], KQ_DT, tag="kT")     # kT[p, dc, s] = K[s, dc*128+p]
        vv = persist.tile([128, N_ST, D], MM_DT, tag="v")      # vv[p, i, e] = V[i*128+p, e]

        # ================= phase A: K/V projections =================
        # x-transposes run two row-groups ahead of the projection matmuls.
        xload = tc.alloc_tile_pool(name="xload", bufs=16)
        xTgp = tc.alloc_tile_pool(name="xTg", bufs=3)
        ps_tr = tc.alloc_tile_pool(name="ps_tr", bufs=3, space="PSUM")
        ps_proj = tc.alloc_tile_pool(name="ps_proj", bufs=3, space="PSUM")

        def emit_xT(src_dram, rg, tagpfx="", preloaded=None, pt_tag="pt"):
            xTg = xTgp.tile([128, N_DC, 512], MM_DT, tag="xTg", name=f"{tagpfx}xTg{rg}")
            for rt in range(4):
                if preloaded is not None:
                    xt = preloaded[rt]
                else:
                    xt = xload.tile([128, C], F32, tag="xt", name=f"{tagpfx}xt{rg}_{rt}")
                    nc.sync.dma_start(xt[:], src_dram[(rg * 4 + rt) * 128 : (rg * 4 + rt + 1) * 128, :])
                pt = ps_tr.tile([128, 512], F32, tag=pt_tag, name=f"{tagpfx}pt{rg}_{rt}")
                for dc in range(N_DC):
                    nc.tensor.transpose(pt[:, dc * 128 : (dc + 1) * 128],
                                        xt[:, dc * 128 : (dc + 1) * 128], identity[:])
                nc.vector.tensor_copy(
                    xTg[:, :, rt * 128 : rt * 128 + 128],
                    pt[:].rearrange("p (a b) -> p a b", a=N_DC),
                )
            return xTg

        # pre-load the ACT exp table during phase-A DMA waits
        warm = const.tile([1, 1], F32, tag="warm")
        nc.scalar.activation(warm[:], ones_f32[0:1, 0:1], ActFn.Exp, scale=1.0)

        pipe = [emit_xT(x, 0)]

        # prefetch the first query block's rows so phase B starts instantly
        xq0_tiles = []
        for rt in range(4):
            xt = xqload.tile([128, C], F32, tag="xqt", name=f"pre_xqt{rt}")
            nc.sync.dma_start(xt[:], xq[rt * 128 : (rt + 1) * 128, :])
            xq0_tiles.append(xt)

        # remaining constants + weights (queue behind the first x loads)
        emit_weight("Wv")
        ones_r = const.tile([128, 128], MM_DT, tag="ones_r")
        nc.vector.tensor_copy(ones_r[:], ones_f32[:])
        bqT = const.tile([128, N_DC], F32, tag="bqT")
        bkT = const.tile([128, N_DC], F32, tag="bkT")
        for g in range(N_DC):
            nc.sync.dma_start(bqT[:, g : g + 1], b_dram["bq"][g * 128 : (g + 1) * 128].unsqueeze(1))
            nc.sync.dma_start(bkT[:, g : g + 1], b_dram["bk"][g * 128 : (g + 1) * 128].unsqueeze(1))
        bias_bc = const.tile([128, 2 * D], F32, tag="bias_bc")
        with tc.tile_pool(name="browp", bufs=1) as browp, \
             tc.tile_pool(name="bcast_ps", bufs=1, space="PSUM") as bps:
            brow = browp.tile([1, 2 * D], F32, tag="brow")
            nc.sync.dma_start(brow[0:1, 0:D], b_dram["bv"][:].unsqueeze(0))
            nc.sync.dma_start(brow[0:1, D : 2 * D], b_dram["bo"][:].unsqueeze(0))
            bt = bps.tile([128, 512], F32)
            nc.tensor.matmul(bt[:], ones_f32[0:1, :], brow[0:1, 0:D])
            nc.vector.tensor_copy(bias_bc[:, 0:D], bt[:])
            bt2 = bps.tile([128, 512], F32)
            nc.tensor.matmul(bt2[:], ones_f32[0:1, :], brow[0:1, D : 2 * D])
            nc.vector.tensor_copy(bias_bc[:, D : 2 * D], bt2[:])
        bv_bc = bias_bc[:, 0:D]
        bo_bc = bias_bc[:, D : 2 * D]

        pipe.append(emit_xT(x, 1))

        N_RG = S // 512
        for rg in range(N_RG):              # 8 row groups of 512 rows
            xTg = pipe.pop(0)
            if rg + 2 < N_RG:
                pipe.append(emit_xT(x, rg + 2))
            # kT for these 512 rows
            for g in range(N_DC):
                pk = ps_proj.tile([128, 512], F32, tag="pk")
                for dc in range(N_DC):
                    nc.tensor.matmul(pk[:], wts["Wk"][:, dc, g * 128 : (g + 1) * 128],
                                     xTg[:, dc, :], start=(dc == 0), stop=(dc == N_DC - 1))
                nc.vector.tensor_scalar_add(kT[:, g, rg * 512 : (rg + 1) * 512], pk[:],
                                            bkT[:, g : g + 1])
            # V for these 512 rows
            for rt in range(4):
                pv = ps_proj.tile([128, 512], F32, tag="pk")
                for dc in range(N_DC):
                    nc.tensor.matmul(pv[:], xTg[:, dc, rt * 128 : (rt + 1) * 128],
                                     wts["Wv"][:, dc, :], start=(dc == 0), stop=(dc == N_DC - 1))
                nc.vector.tensor_add(vv[:, rg * 4 + rt, :], pv[:], bv_bc)
            if rg == 0:
                emit_weight("Wq")
                emit_weight("Wo")

        ps_proj.release(); ps_tr.release()
        xTgp.release(); xload.release()
        wraw_pool.release(); wkv_pool.release()

        # ================= phase B: attention =================
        with tc.tile_pool(name="xqTg", bufs=2) as xqTgp, \
             tc.tile_pool(name="qT", bufs=2) as qTp, \
             tc.tile_pool(name="pT", bufs=6) as pTp, \
             tc.tile_pool(name="rl", bufs=2) as rlp, \
             tc.tile_pool(name="attnT", bufs=2) as attnTp, \
             tc.tile_pool(name="osb", bufs=4) as osbp, \
             tc.tile_pool(name="ps_at", bufs=4, space="PSUM") as ps_atp, \
             tc.tile_pool(name="ps_s", bufs=3, space="PSUM") as ps_sp, \
             tc.tile_pool(name="ps_l", bufs=1, space="PSUM") as ps_lp:

            def emit_qproj(qb, preloaded=None):
                # Q projection for one 512-query block
                xqTg = emit_xT(xq, qb, tagpfx="q", preloaded=preloaded, pt_tag="ss")
                qT = qTp.tile([128, N_DC, 512], KQ_DT, tag="qT", name=f"qT{qb}")
                for g in range(N_DC):
                    pq = ps_sp.tile([128, 512], F32, tag="ss", name=f"pq{qb}_{g}")
                    for dc in range(N_DC):
                        nc.tensor.matmul(pq[:], wts["Wq"][:, dc, g * 128 : (g + 1) * 128],
                                         xqTg[:, dc, :], start=(dc == 0), stop=(dc == N_DC - 1))
                    nc.vector.tensor_scalar_add(qT[:, g, :], pq[:], bqT[:, g : g + 1])
                return qT

            def emit_score(qb, st, qT):
                ss = ps_sp.tile([128, 512], F32, tag="ss", name=f"ss{qb}_{st}")
                for dc in range(N_DC):
                    nc.tensor.matmul(ss[:], kT[:, dc, st * 128 : (st + 1) * 128],
                                     qT[:, dc, :], start=(dc == 0), stop=(dc == N_DC - 1))
                return ss

            # phase-B pools replace phase-A ones; re-point emit_xT pools
            xload = xqload
            xTgp = xqTgp
            ps_tr = ps_sp

            qT_cur = emit_qproj(0, preloaded=xq0_tiles)
            for qb in range(N_QB):
                qT = qT_cur
                l_ps = ps_lp.tile([128, 512], F32, tag="l", name=f"l{qb}")
                at_ps = [ps_atp.tile([128, 512], F32, tag="at", name=f"at{qb}_{et}")
                         for et in range(4)]
                ss_q = [emit_score(qb, 0, qT), emit_score(qb, 1, qT)]
                for st in range(N_ST):
                    if st + 2 < N_ST:
                        ss_q.append(emit_score(qb, st + 2, qT))
                    ss = ss_q.pop(0)
                    pT = pTp.tile([128, 512], MM_DT, tag="pT", name=f"pT{qb}_{st}")
                    nc.scalar.activation(pT[:], ss[:], ActFn.Exp, scale=SCALE)
                    nc.tensor.matmul(l_ps[:], ones_r[:], pT[:],
                                     start=(st == 0), stop=(st == N_ST - 1))
                    for et in range(4):
                        nc.tensor.matmul(at_ps[et][:], vv[:, st, et * 128 : (et + 1) * 128],
                                         pT[:], start=(st == 0), stop=(st == N_ST - 1))

                if qb + 1 < N_QB:
                    qT_cur = emit_qproj(qb + 1)

                # --- epilogue: 1/l applied per query row at the copyback ---
                l_row = rlp.tile([1, 512], F32, tag="l_row", name=f"lrow{qb}")
                nc.vector.tensor_copy(l_row[:], l_ps[0:1, :])
                lt_ps = ps_lp.tile([128, 4], F32, tag="l", name=f"lt{qb}")
                for rt in range(4):
                    nc.tensor.matmul(lt_ps[:, rt : rt + 1],
                                     l_row[0:1, rt * 128 : (rt + 1) * 128],
                                     ones_f32[0:1, 0:1])
                rlT = rlp.tile([128, 4], F32, tag="rlT", name=f"rlT{qb}")
                nc.vector.reciprocal(rlT[:], lt_ps[:])

                attnT = attnTp.tile([128, 4, 512], MM_DT, tag="attnT", name=f"attnT{qb}")
                for et in range(4):
                    nc.vector.tensor_copy(attnT[:, et, :], at_ps[et][:])
                for rt in range(4):
                    po = ps_sp.tile([128, 512], F32, tag="ss", name=f"po{qb}_{rt}")
                    for ec in range(4):
                        nc.tensor.matmul(po[:], attnT[:, ec, rt * 128 : (rt + 1) * 128],
                                         wts["Wo"][:, ec, :], start=(ec == 0), stop=(ec == 3))
                    ot = osbp.tile([128, D], F32, tag="ot", name=f"ot{qb}_{rt}")
                    # ot = po * (1/l[row])  [DVE], then + bo  [DVE]
                    nc.vector.tensor_scalar_mul(ot[:], po[:], rlT[:, rt : rt + 1])
                    nc.vector.tensor_add(ot[:], ot[:], bo_bc)
                    nc.sync.dma_start(out[(qb * 4 + rt) * 128 : (qb * 4 + rt + 1) * 128, :], ot[:])

        xqload.release()
        const.release()
        persist.release()

    nc.compile()
    return nc


_NC_CACHE = None


def _get_nc():
    global _NC_CACHE
    if _NC_CACHE is None:
        _NC_CACHE = _build_program()
    return _NC_CACHE


def kernel(**inputs):
    x = np.ascontiguousarray(np.asarray(inputs["x"], dtype=np.float32))
    xt = x.reshape(B, S, C)
    ws = {k: np.ascontiguousarray(np.asarray(inputs[k], dtype=np.float32))
          for k in ("Wq", "Wk", "Wv", "Wo")}
    bs = {k: np.ascontiguousarray(np.asarray(inputs[k], dtype=np.float32))
          for k in ("bq", "bk", "bv", "bo")}

    in_maps = []
    for c in range(N_CORES):
        b, h = divmod(c, 2)
        m = {"x": xt[b], "xq": np.ascontiguousarray(xt[b, h * Q : (h + 1) * Q])}
        m.update(ws)
        m.update(bs)
        in_maps.append(m)

    nc = _get_nc()
    res = run_bass_kernel_spmd(nc, in_maps, core_ids=list(range(N_CORES)))

    out = np.empty((B, S, D), dtype=np.float32)
    for c in range(N_CORES):
        b, h = divmod(c, 2)
        out[b, h * Q : (h + 1) * Q] = res.results[c]["out"]
    return out.reshape(B, 64, 64, D)


# revision 25
# speedup vs baseline: 1.0170x; 1.0080x over previous
"""Trainium2 Bass kernel for single-head attention (B=4, S=4096, C=D=512).

Sharding: 8 cores = 4 batches x 2 query-halves. Each core:
  - projects K/V for its batch's full 4096 keys (duplicated across the
    two cores of a batch pair -- cheap vs. the O(S^2) attention),
  - projects Q for its own 2048 query rows,
  - computes softmax(Q K^T / sqrt(D)) V and the output projection for
    its 2048 rows.
No collectives; pure SPMD with per-core input slices.

On-chip layout notes:
  - All matmul contractions need the contracted dim on partitions, so x
    is transposed on the PE (128x128 transposes) into xT[d, rows].
  - Scores are computed transposed (scoreT[s, q]) so exp(scoreT) feeds
    the attnT matmul directly with no per-tile transposes.
  - Row sums l[q] come from a ones-matmul (lhsT = ones[128,128]) which
    also gives the partition-broadcast layout needed to normalize attnT.
  - Matmuls run in bf16 (1 cycle/row, fast weight loads, half SBUF) with
    fp32 PSUM accumulation; rel err vs the fp32 reference ~5e-3. Set
    ATTN_MM_DT=f32r for ~3e-4 accuracy at ~1.5x the runtime.
  - The s-loop is software-pipelined: score matmuls for key-tile st+1/st+2
    are issued before the exp(st)-consuming matmuls so the in-order PE
    never waits on the ScalarE.
"""

import sys

for _p in ("/opt/trn_rl_repo", "/root/.axon_site/_ro/trn_rl_repo"):
    if _p not in sys.path:
        sys.path.append(_p)

import numpy as np
import concourse.bacc as bacc
import concourse.mybir as mybir
import concourse.tile as tile
from concourse.bass_utils import run_bass_kernel_spmd
from concourse.masks import make_identity

F32 = mybir.dt.float32
F32R = mybir.dt.float32r
BF16 = mybir.dt.bfloat16

# matmul operand dtype: BF16 (1 cyc/row, fast FWL weight loads, half SBUF)
# or F32R (reduced fp32: ~2 cyc/row, 2x accuracy headroom)
import os
_MODE = os.environ.get("ATTN_MM_DT", "bf16")
# MM_DT: dtype for V/pT/attnT/weights; KQ_DT: dtype for the Q/K tensors
# feeding the score matmul (exp amplifies score error, so it gets priority
# for extra precision in "mixed" mode).
MM_DT = {"bf16": BF16, "f32r": F32R, "mixed": BF16}[_MODE]
KQ_DT = {"bf16": BF16, "f32r": F32R, "mixed": F32R}[_MODE]

B, S, C, D = 4, 4096, 512, 512
Q = S // 2          # queries per core
N_CORES = 8
SCALE = float(D) ** -0.5
QB = 512            # query block (psum bank width in fp32)
N_QB = Q // QB      # 4 query blocks per core
N_ST = S // 128     # 32 key tiles
N_DC = C // 128     # 4 contraction chunks


def _build_program():
    nc = bacc.Bacc(None, target_bir_lowering=False, debug=False)

    x = nc.dram_tensor("x", [S, C], F32, kind="ExternalInput")
    xq = nc.dram_tensor("xq", [Q, C], F32, kind="ExternalInput")
    w_dram = {
        name: nc.dram_tensor(name, [C, D], F32, kind="ExternalInput")
        for name in ("Wq", "Wk", "Wv", "Wo")
    }
    b_dram = {
        name: nc.dram_tensor(name, [D], F32, kind="ExternalInput")
        for name in ("bq", "bk", "bv", "bo")
    }
    out = nc.dram_tensor("out", [Q, D], F32, kind="ExternalOutput")

    ActFn = mybir.ActivationFunctionType

    with tile.TileContext(nc) as tc:
        persist = tc.alloc_tile_pool(name="persist", bufs=1)
        const = tc.alloc_tile_pool(name="const", bufs=1)
        xqload = tc.alloc_tile_pool(name="xqload", bufs=8)
        wkv_pool = tc.alloc_tile_pool(name="wkv", bufs=1)
        wraw_pool = tc.alloc_tile_pool(name="wraw", bufs=2)

        # ---- constants needed by the first transposes ----
        identity = const.tile([128, 128], F32, tag="identity")
        make_identity(nc, identity[:])
        ones_f32 = const.tile([128, 128], F32, tag="ones_f32")
        nc.vector.memset(ones_f32[:], 1.0)

        wts = {}

        def emit_weight(name):
            pool = wkv_pool if name in ("Wk", "Wv") else persist
            wt = pool.tile([128, N_DC, D], MM_DT, tag=f"w_{name}", name=f"w_{name}")
            for dc in range(N_DC):
                raw = wraw_pool.tile([128, D], F32, tag="raw", name=f"raw_{name}{dc}")
                nc.sync.dma_start(raw[:], w_dram[name][dc * 128 : (dc + 1) * 128, :])
                nc.vector.tensor_copy(wt[:, dc, :], raw[:])
            wts[name] = wt

        # DMA priority: Wk, then the first x row-groups, then everything else.
        emit_weight("Wk")

        # ---- persistent activations ----
        kT = persist.tile([128, N_DC, S], KQ_DT, tag="kT")     # kT[p, dc, s] = K[s, dc*128+p]
        vv = persist.tile([128, N_ST, D], MM_DT, tag="v")      # vv[p, i, e] = V[i*128+p, e]

        # ================= phase A: K/V projections =================
        # x-transposes run two row-groups ahead of the projection matmuls.
        xload = tc.alloc_tile_pool(name="xload", bufs=12)
        xTgp = tc.alloc_tile_pool(name="xTg", bufs=3)
        ps_tr = tc.alloc_tile_pool(name="ps_tr", bufs=3, space="PSUM")
        ps_proj = tc.alloc_tile_pool(name="ps_proj", bufs=3, space="PSUM")

        def emit_xT(src_dram, rg, tagpfx="", preloaded=None, pt_tag="pt"):
            xTg = xTgp.tile([128, N_DC, 512], MM_DT, tag="xTg", name=f"{tagpfx}xTg{rg}")
            for rt in range(4):
                if preloaded is not None:
                    xt = preloaded[rt]
                else:
                    xt = xload.tile([128, C], F32, tag="xt", name=f"{tagpfx}xt{rg}_{rt}")
                    nc.sync.dma_start(xt[:], src_dram[(rg * 4 + rt) * 128 : (rg * 4 + rt + 1) * 128, :])
                pt = ps_tr.tile([128, 512], F32, tag=pt_tag, name=f"{tagpfx}pt{rg}_{rt}")
                for dc in range(N_DC):
                    nc.tensor.transpose(pt[:, dc * 128 : (dc + 1) * 128],
                                        xt[:, dc * 128 : (dc + 1) * 128], identity[:])
                nc.vector.tensor_copy(
                    xTg[:, :, rt * 128 : rt * 128 + 128],
                    pt[:].rearrange("p (a b) -> p a b", a=N_DC),
                )
            return xTg

        # pre-load the ACT exp table during phase-A DMA waits
        warm = const.tile([1, 1], F32, tag="warm")
        nc.scalar.activation(warm[:], ones_f32[0:1, 0:1], ActFn.Exp, scale=1.0)

        pipe = [emit_xT(x, 0)]

        # prefetch the first query block's rows so phase B starts instantly
        xq0_tiles = []
        for rt in range(4):
            xt = xqload.tile([128, C], F32, tag="xqt", name=f"pre_xqt{rt}")
            nc.sync.dma_start(xt[:], xq[rt * 128 : (rt + 1) * 128, :])
            xq0_tiles.append(xt)

        # remaining constants + weights (queue behind the first x loads)
        emit_weight("Wv")
        ones_r = const.tile([128, 128], MM_DT, tag="ones_r")
        nc.vector.tensor_copy(ones_r[:], ones_f32[:])
        bqT = const.tile([128, N_DC], F32, tag="bqT")
        bkT = const.tile([128, N_DC], F32, tag="bkT")
        for g in range(N_DC):
            nc.sync.dma_start(bqT[:, g : g + 1], b_dram["bq"][g * 128 : (g + 1) * 128].unsqueeze(1))
            nc.sync.dma_start(bkT[:, g : g + 1], b_dram["bk"][g * 128 : (g + 1) * 128].unsqueeze(1))
        bias_bc = const.tile([128, 2 * D], F32, tag="bias_bc")
        with tc.tile_pool(name="browp", bufs=1) as browp, \
             tc.tile_pool(name="bcast_ps", bufs=1, space="PSUM") as bps:
            brow = browp.tile([1, 2 * D], F32, tag="brow")
            nc.sync.dma_start(brow[0:1, 0:D], b_dram["bv"][:].unsqueeze(0))
            nc.sync.dma_start(brow[0:1, D : 2 * D], b_dram["bo"][:].unsqueeze(0))
            bt = bps.tile([128, 512], F32)
            nc.tensor.matmul(bt[:], ones_f32[0:1, :], brow[0:1, 0:D])
            nc.vector.tensor_copy(bias_bc[:, 0:D], bt[:])
            bt2 = bps.tile([128, 512], F32)
            nc.tensor.matmul(bt2[:], ones_f32[0:1, :], brow[0:1, D : 2 * D])
            nc.vector.tensor_copy(bias_bc[:, D : 2 * D], bt2[:])
        bv_bc = bias_bc[:, 0:D]
        bo_bc = bias_bc[:, D : 2 * D]

        pipe.append(emit_xT(x, 1))

        N_RG = S // 512
        for rg in range(N_RG):              # 8 row groups of 512 rows
            xTg = pipe.pop(0)
            if rg + 2 < N_RG:
                pipe.append(emit_xT(x, rg + 2))
            # kT for these 512 rows
            for g in range(N_DC):
                pk = ps_proj.tile([128, 512], F32, tag="pk")
                for dc in range(N_DC):
                    nc.tensor.matmul(pk[:], wts["Wk"][:, dc, g * 128 : (g + 1) * 128],
                                     xTg[:, dc, :], start=(dc == 0), stop=(dc == N_DC - 1))
                nc.vector.tensor_scalar_add(kT[:, g, rg * 512 : (rg + 1) * 512], pk[:],
                                            bkT[:, g : g + 1])
            # V for these 512 rows
            for rt in range(4):
                pv = ps_proj.tile([128, 512], F32, tag="pk")
                for dc in range(N_DC):
                    nc.tensor.matmul(pv[:], xTg[:, dc, rt * 128 : (rt + 1) * 128],
                                     wts["Wv"][:, dc, :], start=(dc == 0), stop=(dc == N_DC - 1))
                nc.vector.tensor_add(vv[:, rg * 4 + rt, :], pv[:], bv_bc)
            if rg == 0:
                emit_weight("Wq")
                emit_weight("Wo")

        ps_proj.release(); ps_tr.release()
        xTgp.release(); xload.release()
        wraw_pool.release(); wkv_pool.release()

        # ================= phase B: attention =================
        with tc.tile_pool(name="xqTg", bufs=2) as xqTgp, \
             tc.tile_pool(name="qT", bufs=2) as qTp, \
             tc.tile_pool(name="pT", bufs=6) as pTp, \
             tc.tile_pool(name="rl", bufs=2) as rlp, \
             tc.tile_pool(name="attnT", bufs=2) as attnTp, \
             tc.tile_pool(name="osb", bufs=4) as osbp, \
             tc.tile_pool(name="ps_at", bufs=4, space="PSUM") as ps_atp, \
             tc.tile_pool(name="ps_s", bufs=3, space="PSUM") as ps_sp, \
             tc.tile_pool(name="ps_l", bufs=1, space="PSUM") as ps_lp:

            def emit_qproj(qb, preloaded=None):
                # Q projection for one 512-query block
                xqTg = emit_xT(xq, qb, tagpfx="q", preloaded=preloaded, pt_tag="ss")
                qT = qTp.tile([128, N_DC, 512], KQ_DT, tag="qT", name=f"qT{qb}")
                for g in range(N_DC):
                    pq = ps_sp.tile([128, 512], F32, tag="ss", name=f"pq{qb}_{g}")
                    for dc in range(N_DC):
                        nc.tensor.matmul(pq[:], wts["Wq"][:, dc, g * 128 : (g + 1) * 128],
                                         xqTg[:, dc, :], start=(dc == 0), stop=(dc == N_DC - 1))
                    nc.vector.tensor_scalar_add(qT[:, g, :], pq[:], bqT[:, g : g + 1])
                return qT

            def emit_score(qb, st, qT):
                ss = ps_sp.tile([128, 512], F32, tag="ss", name=f"ss{qb}_{st}")
                for dc in range(N_DC):
                    nc.tensor.matmul(ss[:], kT[:, dc, st * 128 : (st + 1) * 128],
                                     qT[:, dc, :], start=(dc == 0), stop=(dc == N_DC - 1))
                return ss

            # phase-B pools replace phase-A ones; re-point emit_xT pools
            xload = xqload
            xTgp = xqTgp
            ps_tr = ps_sp

            qT_cur = emit_qproj(0, preloaded=xq0_tiles)
            for qb in range(N_QB):
                qT = qT_cur
                l_ps = ps_lp.tile([128, 512], F32, tag="l", name=f"l{qb}")
                at_ps = [ps_atp.tile([128, 512], F32, tag="at", name=f"at{qb}_{et}")
                         for et in range(4)]
                ss_q = [emit_score(qb, 0, qT), emit_score(qb, 1, qT)]
                for st in range(N_ST):
                    if st + 2 < N_ST:
                        ss_q.append(emit_score(qb, st + 2, qT))
                    ss = ss_q.pop(0)
                    pT = pTp.tile([128, 512], MM_DT, tag="pT", name=f"pT{qb}_{st}")
                    nc.scalar.activation(pT[:], ss[:], ActFn.Exp, scale=SCALE)
                    nc.tensor.matmul(l_ps[:], ones_r[:], pT[:],
                                     start=(st == 0), stop=(st == N_ST - 1))
                    for et in range(4):
                        nc.tensor.matmul(at_ps[et][:], vv[:, st, et * 128 : (et + 1) * 128],
                                         pT[:], start=(st == 0), stop=(st == N_ST - 1))

                if qb + 1 < N_QB:
                    qT_cur = emit_qproj(qb + 1)

                # --- epilogue: 1/l applied per query row at the copyback ---
                l_row = rlp.tile([1, 512], F32, tag="l_row", name=f"lrow{qb}")
                nc.vector.tensor_copy(l_row[:], l_ps[0:1, :])
                lt_ps = ps_lp.tile([128, 4], F32, tag="l", name=f"lt{qb}")
                for rt in range(4):
                    nc.tensor.matmul(lt_ps[:, rt : rt + 1],
                                     l_row[0:1, rt * 128 : (rt + 1) * 128],
                                     ones_f32[0:1, 0:1])
                rlT = rlp.tile([128, 4], F32, tag="rlT", name=f"rlT{qb}")
                nc.vector.reciprocal(rlT[:], lt_ps[:])

                attnT = attnTp.tile([128, 4, 512], MM_DT, tag="attnT", name=f"attnT{qb}")
                for et in range(4):
                    nc.vector.tensor_copy(attnT[:, et, :], at_ps[et][:])
                for rt in range(4):
                    po = ps_sp.tile([128, 512], F32, tag="ss", name=f"po{qb}_{rt}")
                    for ec in range(4):
                        nc.tensor.matmul(po[:], attnT[:, ec, rt * 128 : (rt + 1) * 128],
                                         wts["Wo"][:, ec, :], start=(ec == 0), stop=(ec == 3))
                    ot = osbp.tile([128, D], F32, tag="ot", name=f"ot{qb}_{rt}")
                    # ot = po * (1/l[row])  [DVE], then + bo  [DVE]
                    nc.vector.tensor_scalar_mul(ot[:], po[:], rlT[:, rt : rt + 1])
                    nc.vector.tensor_add(ot[:], ot[:], bo_bc)
                    nc.sync.dma_start(out[(qb * 4 + rt) * 128 : (qb * 4 + rt + 1) * 128, :], ot[:])

        xqload.release()
        const.release()
        persist.release()

    nc.compile()
    return nc


_NC_CACHE = None


def _get_nc():
    global _NC_CACHE
    if _NC_CACHE is None:
        _NC_CACHE = _build_program()
    return _NC_CACHE


def kernel(**inputs):
    x = np.ascontiguousarray(np.asarray(inputs["x"], dtype=np.float32))
    xt = x.reshape(B, S, C)
    ws = {k: np.ascontiguousarray(np.asarray(inputs[k], dtype=np.float32))
          for k in ("Wq", "Wk", "Wv", "Wo")}
    bs = {k: np.ascontiguousarray(np.asarray(inputs[k], dtype=np.float32))
          for k in ("bq", "bk", "bv", "bo")}

    in_maps = []
    for c in range(N_CORES):
        b, h = divmod(c, 2)
        m = {"x": xt[b], "xq": np.ascontiguousarray(xt[b, h * Q : (h + 1) * Q])}
        m.update(ws)
        m.update(bs)
        in_maps.append(m)

    nc = _get_nc()
    try:
        res = run_bass_kernel_spmd(nc, in_maps, core_ids=list(range(N_CORES)))
    except Exception:
        # transient NRT/device hiccups recover on retry
        import time
        time.sleep(15)
        res = run_bass_kernel_spmd(nc, in_maps, core_ids=list(range(N_CORES)))

    out = np.empty((B, S, D), dtype=np.float32)
    for c in range(N_CORES):
        b, h = divmod(c, 2)
        out[b, h * Q : (h + 1) * Q] = res.results[c]["out"]
    return out.reshape(B, 64, 64, D)
